# revision 14
# baseline (speedup 1.0000x reference)
"""Causal self-attention (S=2048, D=1024, 16 heads x 64) on 8 Trainium2 cores.

Tensor-parallel sharding: 2 heads per core. Each core computes
  qkv_local = x @ Wqkv[:, local]      (local q/k/v columns, q pre-scaled 1/8)
  attn_h    = softmax(mask(q_h k_h^T)) v_h          for its 2 heads
  partial   = concat(attn) @ Wout[local_rows, :]    (128 rows of Wout)
and the host sums the 8 partials (+bias). All matmul inputs are bf16
(fp32 PSUM accumulation); partials are written bf16 and summed in f64.

On-chip layout: q^T is ONE [128, S] tile holding both heads (head-dim on
partitions, h0 rows 0-63, h1 rows 64-127). k^T is two complementary
zero-padded [128, S] tiles (kT[0] = [k_h0; 0], kT[1] = [0; k_h1]) so each
head's logit matmul contracts the full K=128 against the shared q tile --
the other head's q rows hit zero weights. Logits are computed transposed
([key, query]) so exp(logits) feeds the probs@v matmul directly as the
moving operand; v carries an appended ones-column so the same
accumulation also produces the softmax row-sums. No max-subtraction:
logits are ~N(0,1) after the 1/8 scale. Causal masking: off-diagonal
key-blocks are entirely valid (no mask); the four diagonal sub-blocks are
narrowed to surviving query columns and masked with two merged mask
multiplies on GpSimd. v natural layout comes from SBUF->SBUF XBAR DMA
transposes of v^T (no PE transposes, no DVE copies).

Schedule per 512-query chunk ic: logits(h0), logits(h1) [PE] -- the h1
matmuls hide h0's exp [ACT] latency -- then the two probs@v chains, the
next chunk's q/k/v production (hides the rowsum->reciprocal->broadcast->
normalize chain), then the output projection of chunk ic.
"""

import ml_dtypes
import numpy as np

import concourse.bass as bass
import concourse.mybir as mybir
import concourse.tile as tile
from concourse import bacc
from concourse.bass_utils import run_bass_kernel_spmd

S = 2048
D = 1024
DH = 64
N_CORES = 8

P = 128
NB512 = S // 512  # 512-wide query chunks
NB128 = S // 128  # 128-wide chunks
KO = D // P  # contraction chunks for the projections

F32 = mybir.dt.float32
BF16 = mybir.dt.bfloat16

_compiled = {}


def _emit(nc, tc, mm_dt, xt, w, wout, maskA, maskB, out):
    f32 = F32
    with (
        tc.tile_pool(name="const", bufs=1) as const,
        tc.tile_pool(name="epool", bufs=17) as epool,
        tc.tile_pool(name="opool", bufs=4) as opool,
        tc.tile_pool(name="rcpool", bufs=1) as rcpool,
        tc.tile_pool(name="psmm", bufs=2, space="PSUM") as psmm,
        tc.tile_pool(name="psacc", bufs=2, space="PSUM") as psacc,
    ):
        # xT in 8 half-chunk tiles [P, 4, 512]: fine-grained DMA completion
        # tracking so the first q/k matmuls start after ~0.5MB, not 4MB
        sb_xT = [
            [const.tile([P, KO // 2, 512], mm_dt, name=f"sb_xT{si}_{hf}") for hf in (0, 1)]
            for si in range(NB512)
        ]
        sb_w = const.tile([P, KO, 384], mm_dt, name="sb_w")
        sb_wout = const.tile([P, D], mm_dt, name="sb_wout")
        sb_maskA = const.tile([P, 896], mm_dt, name="sb_maskA")
        sb_maskB = const.tile([P, 384], mm_dt, name="sb_maskB")
        sb_qT = const.tile([P, S], mm_dt, name="sb_qT")  # both heads
        sb_kT = [const.tile([P, S], mm_dt, name=f"sb_kT{h}") for h in (0, 1)]
        # per-head v in natural layout: cols 0-63 v, col 64 ones (for the
        # softmax row-sums); blocks padded to 128 cols so every XBAR DMA
        # transpose destination offset stays 128-element aligned
        sb_vh = [const.tile([P, NB128, P], mm_dt, name=f"sb_v{h}") for h in (0, 1)]
        sb_vT = const.tile([P, S], mm_dt, name="sb_vT")
        sb_attnT = const.tile([P, S], mm_dt, name="sb_attnT")

        # weights first on both queues so the first matmul's stationary
        # operand is resident early; xT halves follow, si-cascaded
        for o in range(KO):
            weng = nc.scalar if o % 2 == 0 else nc.gpsimd
            weng.dma_start(sb_w[:, o, :], w[o * P : (o + 1) * P, :])
        xt3 = xt.rearrange("(o p) s -> p o s", p=P)
        for si in range(NB512):
            sl = slice(si * 512, (si + 1) * 512)
            eng = nc.sync if si % 2 == 0 else nc.scalar
            for hf in (0, 1):
                for oo in (0, 2):
                    o0 = hf * 4 + oo
                    eng.dma_start(
                        sb_xT[si][hf][:, oo : oo + 2, :], xt3[:, o0 : o0 + 2, sl]
                    )
        nc.gpsimd.dma_start(sb_maskA[:], maskA[:])
        nc.gpsimd.dma_start(sb_maskB[:], maskB[:])
        nc.gpsimd.dma_start(sb_wout[:], wout[:])
        nc.gpsimd.memset(sb_vh[0][:, :, DH : DH + 1], 1.0)
        nc.gpsimd.memset(sb_vh[1][:, :, DH : DH + 1], 1.0)
        nc.gpsimd.memset(sb_kT[0][DH:P, :], 0.0)
        nc.gpsimd.memset(sb_kT[1][0:DH, :], 0.0)

        def _acc8(ps, wcols, si):
            for o in range(KO):
                nc.tensor.matmul(
                    ps[:],
                    sb_w[:, o, wcols],
                    sb_xT[si][o // 4][:, o % 4, :],
                    start=(o == 0),
                    stop=(o == KO - 1),
                )

        # q^T / k^T producer: [c, s] = sum_D W[D, c] * xT[D, s]
        def emit_qk(si):
            sl = slice(si * 512, (si + 1) * 512)
            ps = psmm.tile([P, 512], f32, name="ps_q", tag="mm")
            _acc8(ps, slice(0, 128), si)
            nc.vector.tensor_copy(sb_qT[:, sl], ps[:])
            ps2 = psmm.tile([P, 512], f32, name="ps_k", tag="mm")
            _acc8(ps2, slice(128, 256), si)
            nc.vector.tensor_copy(sb_kT[0][0:DH, sl], ps2[0:DH, :])
            nc.vector.tensor_copy(sb_kT[1][DH:P, sl], ps2[DH:P, :])

        # v^T producer, then XBAR DMA transposes into per-head natural v
        def emit_vT(si):
            sl = slice(si * 512, (si + 1) * 512)
            psv = psmm.tile([P, 512], f32, name="ps_vT", tag="mm")
            _acc8(psv, slice(256, 384), si)
            nc.vector.tensor_copy(sb_vT[:, sl], psv[:])

        def emit_v(sc):
            blk = sb_vT[:, sc * P : (sc + 1) * P]
            eng = nc.sync if sc % 2 == 0 else nc.scalar
            eng.dma_start_transpose(sb_vh[0][:, sc, 0:DH], blk[0:DH, :])
            eng.dma_start_transpose(sb_vh[1][:, sc, 0:DH], blk[DH:P, :])

        # logit groups for one (ic, h): off-diagonal pairs need no mask;
        # diagonal sub-blocks narrowed to surviving query columns, masked
        # with one merged multiply per exp tile on GpSimd
        def emit_logits(ic, h, es):
            groups = []  # [(jc, col_start, n, i0), ...] ; mask tile per group
            for jp in range(2 * ic):
                groups.append(
                    ([(2 * jp, 0, 512, 0), (2 * jp + 1, 512, 512, 0)], None)
                )
            groups.append(
                ([(4 * ic, 0, 512, 0), (4 * ic + 1, 512, 384, 128)], sb_maskA)
            )
            groups.append(
                ([(4 * ic + 2, 0, 256, 256), (4 * ic + 3, 256, 128, 384)], sb_maskB)
            )
            for grp, msk in groups:
                tot = grp[-1][1] + grp[-1][2]
                pl = psmm.tile([P, 1024], f32, name="ps_l", tag="mm2")
                for jc, cs, n, i0 in grp:
                    nc.tensor.matmul(
                        pl[:, cs : cs + n],
                        sb_kT[h][:, jc * P : (jc + 1) * P],
                        sb_qT[:, ic * 512 + i0 : ic * 512 + i0 + n],
                        start=True,
                        stop=True,
                    )
                e = epool.tile([P, 1024], mm_dt, name="e_t", tag="e")
                nc.scalar.activation(
                    e[:, :tot], pl[:, :tot], mybir.ActivationFunctionType.Exp
                )
                if msk is not None:
                    nc.gpsimd.tensor_mul(e[:, :tot], e[:, :tot], msk[:, :tot])
                for jc, cs, n, i0 in grp:
                    es[(h, jc)] = (e[:, cs : cs + n], i0, n)

        # probs@v chain + normalization for one (ic, h)
        def emit_pv(ic, h, es):
            njc = 4 * (ic + 1)
            acc = psacc.tile([DH + 1, 512], f32, name="ps_acc", tag="acc")
            for jc in range(njc):
                eh, i0, n = es[(h, jc)]
                nc.tensor.matmul(
                    acc[:, i0 : i0 + n],
                    sb_vh[h][:, jc, 0 : DH + 1],
                    eh,
                    start=(jc == 0),
                    stop=(jc == njc - 1),
                )
            rsk = rcpool.tile([1, 512], f32, name="rsk", tag="rsk", bufs=2)
            nc.vector.tensor_copy(rsk[:], acc[DH : DH + 1, :])
            rck = rcpool.tile([1, 512], f32, name="rck", tag="rck", bufs=3)
            nc.vector.reciprocal_approx_fast(rck[:], rsk[:])
            bck = rcpool.tile([DH, 512], f32, name="bck", tag="bck", bufs=3)
            nc.gpsimd.partition_broadcast(bck[:], rck[:])
            po = h * DH
            dst = sb_attnT[po : po + DH, ic * 512 : (ic + 1) * 512]
            nc.vector.tensor_mul(dst, acc[0:DH, :], bck[:])

        # output projection for one 128-row query chunk
        def emit_proj(sc):
            for ec in range(D // 512):
                pp = psmm.tile([P, 512], f32, name="ps_p", tag="mm")
                nc.tensor.matmul(
                    pp[:],
                    sb_attnT[:, sc * P : (sc + 1) * P],
                    sb_wout[:, ec * 512 : (ec + 1) * 512],
                    start=True,
                    stop=True,
                )
                ot = opool.tile([P, 512], out.dtype, name="ot", tag="ot")
                if (sc * 2 + ec) % 2 == 0:
                    nc.scalar.copy(ot[:], pp[:])
                else:
                    nc.vector.tensor_copy(ot[:], pp[:])
                eng = nc.sync if (sc * 2 + ec) % 2 == 0 else nc.scalar
                eng.dma_start(
                    out[sc * P : (sc + 1) * P, ec * 512 : (ec + 1) * 512], ot[:]
                )

        emit_qk(0)
        emit_vT(0)
        for sc in range(4):
            emit_v(sc)
        for ic in range(NB512):
            es = {}
            emit_logits(ic, 0, es)
            emit_logits(ic, 1, es)
            emit_pv(ic, 0, es)
            emit_pv(ic, 1, es)
            if ic + 1 < NB512:
                emit_qk(ic + 1)
                emit_vT(ic + 1)
                for sc in range(4 * (ic + 1), 4 * (ic + 2)):
                    emit_v(sc)
            for sc in range(4 * ic, 4 * (ic + 1)):
                emit_proj(sc)


def build(mm_dt=BF16, out_dt=BF16):
    key = (str(mm_dt), str(out_dt))
    if key in _compiled:
        return _compiled[key]
    nc = bacc.Bacc("TRN2", target_bir_lowering=False, debug=False, num_devices=N_CORES)
    xt = nc.dram_tensor("xt", [D, S], mm_dt, kind="ExternalInput").ap()
    w = nc.dram_tensor("w", [D, 384], mm_dt, kind="ExternalInput").ap()
    wout = nc.dram_tensor("wout", [P, D], mm_dt, kind="ExternalInput").ap()
    maskA = nc.dram_tensor("maskA", [P, 896], mm_dt, kind="ExternalInput").ap()
    maskB = nc.dram_tensor("maskB", [P, 384], mm_dt, kind="ExternalInput").ap()
    out = nc.dram_tensor("out", [S, D], out_dt, kind="ExternalOutput").ap()
    with tile.TileContext(nc) as tc:
        _emit(nc, tc, mm_dt, xt, w, wout, maskA, maskB, out)
    nc.compile()
    _compiled[key] = nc
    return nc


def _np_dt(mm_dt):
    return ml_dtypes.bfloat16 if mm_dt == BF16 else np.float32


def make_inputs(x, Wqkv, Wout, mm_dt=BF16):
    """Host-side shard/layout prep -> per-core input maps."""
    np_dt = _np_dt(mm_dt)
    x = np.ascontiguousarray(np.asarray(x, np.float32))
    Wqkv = np.asarray(Wqkv, np.float32)
    Wout = np.asarray(Wout, np.float32)
    xT = np.ascontiguousarray(x.T.astype(np_dt))  # [D, S]
    p = np.arange(P)[:, None]
    fA = np.arange(896)[None, :]
    maskA = np.where(fA < 512, p <= fA, p <= fA - 512).astype(np_dt)
    fB = np.arange(384)[None, :]
    maskB = np.where(fB < 256, p <= fB, p <= fB - 256).astype(np_dt)
    in_maps = []
    for c in range(N_CORES):
        wq = Wqkv[:, 128 * c : 128 * (c + 1)] * (1.0 / np.sqrt(DH))
        wk = Wqkv[:, D + 128 * c : D + 128 * (c + 1)]
        wv = Wqkv[:, 2 * D + 128 * c : 2 * D + 128 * (c + 1)]
        w_loc = np.ascontiguousarray(np.concatenate([wq, wk, wv], axis=1))
        wout_loc = np.ascontiguousarray(Wout[128 * c : 128 * (c + 1), :]).astype(np_dt)
        in_maps.append(
            {
                "xt": xT,
                "w": w_loc.astype(np_dt),
                "wout": wout_loc,
                "maskA": np.ascontiguousarray(maskA),
                "maskB": np.ascontiguousarray(maskB),
            }
        )
    return in_maps


def kernel(x, Wqkv, Wout, bias, mm_dt=BF16, **run_kwargs):
    nc = build(mm_dt)
    in_maps = make_inputs(x, Wqkv, Wout, mm_dt)
    res = run_bass_kernel_spmd(nc, in_maps, core_ids=list(range(N_CORES)), **run_kwargs)
    acc = np.zeros((S, D), np.float64)
    for c in range(N_CORES):
        acc += res.results[c]["out"].astype(np.float64)
    acc += np.asarray(bias, np.float64)[None, :]
    return acc.astype(np.float32)
